# revision 1
# baseline (speedup 1.0000x reference)
"""Trainium2 Bass kernel for nn_BasicBlock_1w1a (binary conv BasicBlock).

Self-contained: takes FULL inputs (batch 64), shards batch across 8 NeuronCores,
runs a single SPMD Bass/Tile kernel with in-kernel AllReduces for the
training-mode BatchNorm batch statistics, gathers the full output.

Per block (twice):
  S      = conv3x3(sign(x), sign(w))        # fp8 DoubleRow matmuls, exact
  gate   = sigmoid(BN_dada(avgpool8(x) @ dw))
  u      = prelu(S * alpha * gate, a)       # fused into PSUM eviction (ACT)
  out    = BN(u) * g + b + x                # batch stats via AllReduce
"""
import os
import sys

sys.path.insert(0, "/opt/trn_rl_repo")

import numpy as np
import ml_dtypes

import concourse.bass as bass
import concourse.bacc as bacc
import concourse.tile as tile
import concourse.mybir as mybir
from concourse import bass_utils

P = 128
CI = 2
NIMG = 8
NCORES = 8
H = W = 32
S = H * W
SP = 34 * 34
CH = 2
EPS = 1e-5
MAGIC = 0x5F3759DF
AF = mybir.ActivationFunctionType
ALU = mybir.AluOpType
DT = mybir.dt
X_AXIS = mybir.AxisListType.X
XY_AXIS = mybir.AxisListType.XY

DEBUG = False
_CACHE = {}


def _build(debug=False):
    nc = bacc.Bacc("TRN2", target_bir_lowering=False, debug=False,
                   num_devices=NCORES)

    x_in = nc.dram_tensor("x", [NIMG, 256, S], DT.float32, kind="ExternalInput")
    w1_in = nc.dram_tensor("w1sb", [P, CI, 9, 2, P], DT.float8e4,
                           kind="ExternalInput")
    w2_in = nc.dram_tensor("w2sb", [P, CI, 9, 2, P], DT.float8e4,
                           kind="ExternalInput")
    # dada weights split hi/lo bf16: [c_lo, ci, hilo, oi, o_lo]
    dw1_in = nc.dram_tensor("dwt1", [P, CI, 2, 2, P], DT.bfloat16,
                            kind="ExternalInput")
    dw2_in = nc.dram_tensor("dwt2", [P, CI, 2, 2, P], DT.bfloat16,
                            kind="ExternalInput")
    # packed per-channel params: j = 0:alpha 1:a 2:g 3:b 4:dg 5:db -> [P, 6, CI]
    pk1_in = nc.dram_tensor("pk1", [P, 6, CI], DT.float32, kind="ExternalInput")
    pk2_in = nc.dram_tensor("pk2", [P, 6, CI], DT.float32, kind="ExternalInput")
    out_t = nc.dram_tensor("out", [NIMG, 256, S], DT.float32,
                           kind="ExternalOutput")

    dbg = {}
    if debug:
        dbg["u1"] = nc.dram_tensor("dbg_u1", [P, 2, NIMG, S], DT.float32,
                                   kind="ExternalOutput")
        dbg["gate1"] = nc.dram_tensor("dbg_gate1", [P, 2, NIMG], DT.float32,
                                      kind="ExternalOutput")
        dbg["p1"] = nc.dram_tensor("dbg_p1", [P, CI, NIMG, 16], DT.float32,
                                   kind="ExternalOutput")
        dbg["ar1"] = nc.dram_tensor("dbg_ar1", [P, 4], DT.float32,
                                    kind="ExternalOutput")
        dbg["ar2"] = nc.dram_tensor("dbg_ar2", [P, 4], DT.float32,
                                    kind="ExternalOutput")
        dbg["x1"] = nc.dram_tensor("dbg_x1", [NIMG, 256, S], DT.float32,
                                   kind="ExternalOutput")

    with tile.TileContext(nc) as tc:
        with tc.tile_pool(name="big", bufs=1) as big, \
             tc.tile_pool(name="small", bufs=1) as small, \
             tc.tile_pool(name="psum", bufs=3, space="PSUM") as psum_pool, \
             tc.tile_pool(name="psum_y", bufs=2, space="PSUM") as psum_y_pool, \
             tc.tile_pool(name="sq", bufs=2) as sqpool, \
             tc.tile_pool(name="tmp", bufs=2) as tmppool, \
             tc.tile_pool(name="poola", bufs=2) as poola_pool, \
             tc.tile_pool(name="dram", bufs=1, space="DRAM") as dram:

            # ---- warmup collective: absorbs ncfw init + SPMD launch skew ----
            wu = small.tile([P, 1], DT.float32, tag="wu")
            nc.gpsimd.memset(wu[:], 1.0)
            wu_i = dram.tile([P, 1], DT.float32, tag="wu_i")
            wu_o = dram.tile([P * NCORES, 1], DT.float32, tag="wu_o")
            nc.sync.dma_start(wu_i[:], wu[:])
            nc.gpsimd.collective_compute(
                "AllGather", ALU.bypass, replica_groups=[list(range(NCORES))],
                ins=[wu_i[:].opt()], outs=[wu_o[:].opt()])

            def allreduce_stats(stat_sb, out_sb, widx, name):
                """AllGather [128,4] partials + deterministic local reduce
                (AG floor ~5us vs AR ~25us)."""
                bi = dram.tile([P, 4], DT.float32, tag=f"bi_{name}{widx}")
                bo = dram.tile([P * NCORES, 4], DT.float32,
                               tag=f"bo_{name}{widx}")
                nc.sync.dma_start(bi[:], stat_sb[:])
                nc.gpsimd.collective_compute(
                    "AllGather", ALU.bypass,
                    replica_groups=[list(range(NCORES))],
                    ins=[bi[:].opt()], outs=[bo[:].opt()])
                gath = small.tile([P, NCORES, 4], DT.float32,
                                  tag=f"gth_{name}{widx}")
                nc.sync.dma_start(
                    gath[:], bo[:].rearrange("(r p) c -> p r c", p=P))
                nc.vector.tensor_reduce(out_sb[:],
                                        gath[:].rearrange("p r c -> p c r"),
                                        axis=X_AXIS, op=ALU.add)

            xt = big.tile([P, NIMG, CI, S], DT.float32, tag="xt")
            ut = big.tile([P, 2, NIMG, S], DT.float32, tag="ut")
            spad = big.tile([P, CI, NIMG, SP], DT.float8e4, tag="spad")
            w1sb = big.tile([P, CI, 9, 2, P], DT.float8e4, tag="w1")
            w2sb = big.tile([P, CI, 9, 2, P], DT.float8e4, tag="w2")
            dwt1 = big.tile([P, CI, 2, 2, P], DT.bfloat16, tag="dwt1")
            dwt2 = big.tile([P, CI, 2, 2, P], DT.bfloat16, tag="dwt2")
            pk1 = big.tile([P, 6, CI], DT.float32, tag="pk1")
            pk2 = big.tile([P, 6, CI], DT.float32, tag="pk2")

            nc.vector.memset(
                spad[:].rearrange("p c n s -> p (c n s)").bitcast(DT.int32), 0)
            def dma_x(n):
                xv = x_in[n].rearrange("(ci p) s -> p ci s", p=P)
                for ci in range(CI):
                    nc.sync.dma_start(xt[:, n, ci, :], xv[:, ci, :])

            for n in (0, 1):
                dma_x(n)
            nc.sync.dma_start(w1sb[:], w1_in[:])
            nc.sync.dma_start(pk1[:], pk1_in[:])
            nc.sync.dma_start(dwt1[:], dw1_in[:])
            for n in range(2, NIMG):
                dma_x(n)
            nc.sync.dma_start(w2sb[:], w2_in[:])
            nc.sync.dma_start(dwt2[:], dw2_in[:])
            nc.sync.dma_start(pk2[:], pk2_in[:])

            def sign_into_spad(n, ci):
                view = spad[:, ci, n, :].rearrange("p (r c) -> p r c", r=34)
                nc.scalar.activation(
                    view[:, 1:33, 1:33],
                    xt[:, n, ci, :].rearrange("p (h w) -> p h w", h=H),
                    AF.Sign)

            def pools_into(p_t, n, ci):
                pa = poola_pool.tile([P, H * 4], DT.float32, tag="poola",
                                     name=f"poola_{n}_{ci}")
                nc.vector.tensor_reduce(
                    pa[:],
                    xt[:, n, ci, :].rearrange("p (h pw w) -> p h pw w",
                                              h=H, pw=4),
                    axis=X_AXIS, op=ALU.add)
                nc.vector.tensor_reduce(
                    p_t[:, ci, n, :].rearrange("p (ph pw) -> p ph pw", ph=4),
                    pa[:].rearrange("p (ph hh pw) -> p ph pw hh", ph=4, hh=8),
                    axis=X_AXIS, op=ALU.add)

            def rsqrt_inplace(k, t, e1):
                """k = 1/sqrt(t), all DVE (quake seed + 3 Newton)."""
                ki = k.bitcast(DT.int32)
                nc.vector.tensor_scalar(ki, t.bitcast(DT.int32), 1, None,
                                        ALU.arith_shift_right)
                nc.vector.tensor_scalar(ki, ki, MAGIC, None, ALU.subtract)
                nc.vector.tensor_scalar(ki, ki, -1, None, ALU.mult)
                for _ in range(3):
                    nc.vector.tensor_mul(e1, k, k)
                    nc.vector.tensor_mul(e1, e1, t)
                    nc.vector.tensor_scalar(e1, e1, -0.5, 1.5, ALU.mult,
                                            ALU.add)
                    nc.vector.tensor_mul(k, k, e1)

            signs_done = set()
            pools_done = set()
            p_tiles = {
                1: small.tile([P, CI, NIMG, 16], DT.float32, name="p_t1",
                              tag="p1"),
                2: small.tile([P, CI, NIMG, 16], DT.float32, name="p_t2",
                              tag="p2"),
            }

            def conv_block(widx, wsb, dwt, pk, last):
                p_t = p_tiles[widx]
                p_t2_next = p_tiles.get(widx + 1)
                ph = small.tile([P, CI, NIMG * 16], DT.bfloat16, tag=f"ph{widx}")
                pl = small.tile([P, CI, NIMG * 16], DT.bfloat16, tag=f"pl{widx}")
                ysb = small.tile([P, 2, NIMG * 16], DT.float32, tag=f"y{widx}")
                m_s = small.tile([P, 2, NIMG], DT.float32, tag=f"ms{widx}")
                m1 = small.tile([P, 2, NIMG], DT.float32, tag=f"m1{widx}")
                gate = small.tile([P, 2, NIMG], DT.float32, tag=f"g{widx}")
                ystat = small.tile([P, 4], DT.float32, tag=f"ys{widx}")
                usum = small.tile([P, 2, NIMG], DT.float32, tag=f"us{widx}")
                usq = small.tile([P, 2, NIMG], DT.float32, tag=f"uq{widx}")
                ustat = small.tile([P, 4], DT.float32, tag=f"ut{widx}")
                ar_y = small.tile([P, 4], DT.float32, tag=f"ary{widx}")
                ar_u = small.tile([P, 4], DT.float32, tag=f"aru{widx}")
                AB = small.tile([P, 2, 2], DT.float32, tag=f"ab{widx}")

                for n in range(NIMG):
                    for ci in range(CI):
                        if (widx, n) not in signs_done:
                            sign_into_spad(n, ci)
                        if (widx, n) not in pools_done:
                            pools_into(p_t, n, ci)
                    signs_done.add((widx, n))
                    pools_done.add((widx, n))

                # hi/lo split of pool sums for exact-ish bf16 dada matmul
                nc.vector.tensor_copy(ph[:], p_t[:].rearrange("p c n s -> p c (n s)"))
                nc.vector.tensor_sub(pl[:],
                                     p_t[:].rearrange("p c n s -> p c (n s)"),
                                     ph[:])

                for oi in range(2):
                    psy = psum_y_pool.tile([P, NIMG * 16], DT.float32,
                                           tag="psy", name=f"psy{widx}_{oi}")
                    terms = [(hl, pp) for hl in range(2) for pp in (ph, pl)]
                    for ci in range(CI):
                        for ti, (hl, pp) in enumerate(terms):
                            nc.tensor.matmul(
                                psy[:], dwt[:, ci, hl, oi, :], pp[:, ci, :],
                                start=(ci == 0 and ti == 0),
                                stop=(ci == CI - 1 and ti == len(terms) - 1))
                    nc.scalar.activation(ysb[:, oi, :], psy[:], AF.Copy,
                                         accum_out=ystat[:, oi:oi + 1])
                    sq = sqpool.tile([P, 512], DT.float32, tag="sq",
                                     name=f"ysq{widx}_{oi}")
                    nc.scalar.activation(sq[:, :NIMG * 16], ysb[:, oi, :],
                                         AF.Square,
                                         accum_out=ystat[:, 2 + oi:3 + oi])
                    nc.vector.tensor_reduce(
                        m_s[:, oi, :],
                        ysb[:, oi, :].rearrange("p (n s) -> p n s", n=NIMG),
                        axis=X_AXIS, op=ALU.add)

                # stats exchange #1 (dada)
                allreduce_stats(ystat, ar_y, widx, "y")

                cnt_y = float(NCORES * NIMG * 16)
                for oi in range(2):
                    t = small.tile([P, 1], DT.float32, tag=f"t{widx}_{oi}")
                    mu = small.tile([P, 1], DT.float32, tag=f"mu{widx}_{oi}")
                    k = small.tile([P, 1], DT.float32, tag=f"k{widx}_{oi}")
                    e1 = small.tile([P, 1], DT.float32, tag=f"e{widx}_{oi}")
                    A = small.tile([P, 1], DT.float32, tag=f"A{widx}_{oi}")
                    B = small.tile([P, 1], DT.float32, tag=f"B{widx}_{oi}")
                    nc.vector.tensor_scalar(t[:], ar_y[:, 2 + oi:3 + oi],
                                            1.0 / cnt_y, EPS, ALU.mult, ALU.add)
                    nc.vector.tensor_scalar(mu[:], ar_y[:, oi:oi + 1],
                                            1.0 / cnt_y, None, ALU.mult)
                    nc.vector.tensor_mul(e1[:], mu[:], mu[:])
                    nc.vector.tensor_sub(t[:], t[:], e1[:])
                    rsqrt_inplace(k[:], t[:], e1[:])
                    nc.vector.tensor_mul(A[:], k[:], pk[:, 4, oi:oi + 1])
                    nc.vector.tensor_mul(B[:], mu[:], A[:])
                    nc.vector.tensor_sub(B[:], pk[:, 5, oi:oi + 1], B[:])
                    nc.vector.tensor_scalar(m1[:, oi, :], m_s[:, oi, :],
                                            1.0 / 16.0, None, ALU.mult)
                    sig = small.tile([P, NIMG], DT.float32,
                                     tag=f"sg{widx}_{oi}")
                    nc.scalar.activation(sig[:], m1[:, oi, :], AF.Sigmoid,
                                         bias=B[:], scale=A[:])
                    nc.vector.tensor_scalar(gate[:, oi, :], sig[:],
                                            pk[:, 0, oi:oi + 1], None, ALU.mult)

                # conv matmuls (fp8 DoubleRow, K=256 per MM) + fused evac
                for n in range(NIMG):
                    sview = spad[:, :, n, :].rearrange("p ci (r c) -> p ci r c",
                                                       r=34)
                    for oi in range(2):
                        ps = psum_pool.tile([P, S], DT.float32, tag="ps",
                                            name=f"ps{widx}_{n}_{oi}")
                        for kk in range(9):
                            dy, dx = kk // 3, kk % 3
                            lhsT = wsb[:, :, kk, oi, :]
                            for c2 in range(CH):
                                nc.tensor.matmul(
                                    ps[:, c2 * 512:(c2 + 1) * 512], lhsT,
                                    sview[:, :, c2 * 16 + dy:c2 * 16 + dy + 16,
                                          dx:dx + 32],
                                    start=(kk == 0), stop=(kk == 8),
                                    perf_mode=mybir.MatmulPerfMode.DoubleRow)
                        # u' = prelu(S, a): NO gate dependency — the gate
                        # (and alpha) fold into the BN affine later since
                        # prelu(g*S, a) = g*prelu(S, a) for g > 0.
                        u_sl = ut[:, oi, n, :]
                        nc.scalar.activation(
                            u_sl, ps[:], AF.Prelu,
                            alpha=pk[:, 1, oi:oi + 1],
                            accum_out=usum[:, oi, n:n + 1])
                        sq = sqpool.tile([P, S], DT.float32, tag="sq",
                                         name=f"sq{widx}_{n}_{oi}")
                        nc.scalar.activation(
                            sq[:], u_sl, AF.Square,
                            accum_out=usq[:, oi, n:n + 1])

                # main BN stats: gate-weighted sums of per-image accums
                # sum(u) = sum_n g'[n]*usum'[n], sum(u^2) = sum_n g'^2[n]*usq'[n]
                for oi in range(2):
                    us8 = small.tile([P, NIMG], DT.float32,
                                     tag=f"us8{widx}_{oi}")
                    nc.vector.tensor_mul(us8[:], usum[:, oi], gate[:, oi, :])
                    nc.vector.tensor_reduce(ustat[:, oi:oi + 1], us8[:],
                                            axis=X_AXIS, op=ALU.add)
                    uq8 = small.tile([P, NIMG], DT.float32,
                                     tag=f"uq8{widx}_{oi}")
                    g2 = small.tile([P, NIMG], DT.float32,
                                    tag=f"g2{widx}_{oi}")
                    nc.vector.tensor_mul(g2[:], gate[:, oi, :], gate[:, oi, :])
                    nc.vector.tensor_mul(uq8[:], usq[:, oi], g2[:])
                    nc.vector.tensor_reduce(ustat[:, 2 + oi:3 + oi], uq8[:],
                                            axis=X_AXIS, op=ALU.add)
                allreduce_stats(ustat, ar_u, widx, "u")

                cnt_u = float(NCORES * NIMG * S)
                for ci in range(2):
                    t = small.tile([P, 1], DT.float32, tag=f"tu{widx}_{ci}")
                    mu = small.tile([P, 1], DT.float32, tag=f"muu{widx}_{ci}")
                    k = small.tile([P, 1], DT.float32, tag=f"ku{widx}_{ci}")
                    e1 = small.tile([P, 1], DT.float32, tag=f"eu{widx}_{ci}")
                    nc.vector.tensor_scalar(t[:], ar_u[:, 2 + ci:3 + ci],
                                            1.0 / cnt_u, EPS, ALU.mult, ALU.add)
                    nc.vector.tensor_scalar(mu[:], ar_u[:, ci:ci + 1],
                                            1.0 / cnt_u, None, ALU.mult)
                    nc.vector.tensor_mul(e1[:], mu[:], mu[:])
                    nc.vector.tensor_sub(t[:], t[:], e1[:])
                    rsqrt_inplace(k[:], t[:], e1[:])
                    nc.vector.tensor_mul(AB[:, 0, ci:ci + 1], k[:],
                                         pk[:, 2, ci:ci + 1])
                    nc.vector.tensor_mul(e1[:], mu[:], AB[:, 0, ci:ci + 1])
                    nc.vector.tensor_sub(AB[:, 1, ci:ci + 1],
                                         pk[:, 3, ci:ci + 1], e1[:])

                # per-image scale gA[n] = A * g'[n] (gate folded in here)
                gA = small.tile([P, 2, NIMG], DT.float32, tag=f"ga{widx}")
                for ci in range(2):
                    nc.vector.tensor_scalar(gA[:, ci, :], gate[:, ci, :],
                                            AB[:, 0, ci:ci + 1], None, ALU.mult)

                # x_out = gA[n]*u' + B + x  (in place over xt).
                # Mid-block: DVE-only affine, immediately followed by the next
                # conv's sign+pools for that image so its matmuls can start.
                # Last block: split scale+bias ACT/DVE to shorten the tail.
                for n in range(NIMG):
                    ov = out_t[n].rearrange("(ci p) s -> p ci s", p=P)
                    for ci in range(CI):
                        tmp = tmppool.tile([P, S], DT.float32, tag="tmp",
                                           name=f"tmp{widx}_{n}_{ci}")
                        if last and n % 2 == 0:
                            nc.scalar.activation(tmp[:], ut[:, ci, n, :],
                                                 AF.Identity,
                                                 bias=AB[:, 1, ci:ci + 1],
                                                 scale=gA[:, ci, n:n + 1])
                        else:
                            nc.vector.tensor_scalar(tmp[:], ut[:, ci, n, :],
                                                    gA[:, ci, n:n + 1],
                                                    AB[:, 1, ci:ci + 1],
                                                    ALU.mult, ALU.add)
                        nc.vector.tensor_add(xt[:, n, ci, :], tmp[:],
                                             xt[:, n, ci, :])
                        if last:
                            nc.sync.dma_start(ov[:, ci, :], xt[:, n, ci, :])
                    if not last:
                        for ci in range(CI):
                            sign_into_spad(n, ci)
                        signs_done.add((widx + 1, n))

                if debug and widx == 1:
                    nc.sync.dma_start(dbg["p1"][:], p_t[:])
                    nc.sync.dma_start(dbg["gate1"][:], gate[:])
                    nc.sync.dma_start(dbg["ar1"][:], ar_y[:])
                    nc.sync.dma_start(dbg["ar2"][:], ar_u[:])
                    nc.sync.dma_start(dbg["u1"][:], ut[:])
                    for n in range(NIMG):
                        nc.sync.dma_start(
                            dbg["x1"][n].rearrange("(ci p) s -> p ci s", p=P),
                            xt[:, n, :, :])

            conv_block(1, w1sb, dwt1, pk1, last=False)
            conv_block(2, w2sb, dwt2, pk2, last=True)

    nc.compile()
    return nc


def _pack_w(w):
    ws = np.sign(w.astype(np.float32))
    t = ws.reshape(2, P, CI, P, 3, 3)           # oi, o_lo, ci, c_lo, dy, dx
    t = t.transpose(3, 2, 4, 5, 0, 1)           # c_lo, ci, dy, dx, oi, o_lo
    return np.ascontiguousarray(t.reshape(P, CI, 9, 2, P)).astype(
        ml_dtypes.float8_e4m3)


def _pack_dw(dw):
    d = (dw.astype(np.float32) / 64.0).reshape(2, P, CI, P)  # oi,o_lo,ci,c_lo
    d = d.transpose(3, 2, 0, 1)                               # c_lo,ci,oi,o_lo
    hi = d.astype(ml_dtypes.bfloat16)
    lo = (d - hi.astype(np.float32)).astype(ml_dtypes.bfloat16)
    out = np.empty((P, CI, 2, 2, P), ml_dtypes.bfloat16)
    out[:, :, 0] = hi
    out[:, :, 1] = lo
    return out


def _pack_pk(w, a, g, b, dg, db):
    alpha = np.abs(w.astype(np.float32)).mean(axis=(1, 2, 3))
    fields = [alpha, a, g, b, dg, db]
    pk = np.empty((P, 6, CI), np.float32)
    for j, f in enumerate(fields):
        pk[:, j, :] = np.asarray(f, np.float32).reshape(CI, P).T
    return pk


def kernel(**inputs):
    key = ("dbg" if DEBUG else "std")
    if key not in _CACHE:
        _CACHE[key] = _build(debug=DEBUG)
    nc = _CACHE[key]

    x = np.asarray(inputs["x"], np.float32).reshape(64, 256, S)
    feed = {
        "w1sb": _pack_w(np.asarray(inputs["w1"])),
        "w2sb": _pack_w(np.asarray(inputs["w2"])),
        "dwt1": _pack_dw(np.asarray(inputs["dw1"])),
        "dwt2": _pack_dw(np.asarray(inputs["dw2"])),
        "pk1": _pack_pk(np.asarray(inputs["w1"]), inputs["a1"], inputs["g1"],
                        inputs["b1"], inputs["dg1"], inputs["db1"]),
        "pk2": _pack_pk(np.asarray(inputs["w2"]), inputs["a2"], inputs["g2"],
                        inputs["b2"], inputs["dg2"], inputs["db2"]),
    }
    in_maps = []
    for c in range(NCORES):
        m = dict(feed)
        m["x"] = np.ascontiguousarray(x[c * NIMG:(c + 1) * NIMG])
        in_maps.append(m)

    trace = bool(int(os.environ.get("BASS_KERNEL_TRACE", "0")))
    res = bass_utils.run_bass_kernel_spmd(
        nc, in_maps, core_ids=list(range(NCORES)), trace=trace)
    kernel.last_results = res

    out = np.concatenate([res.results[c]["out"] for c in range(NCORES)], axis=0)
    return out.reshape(64, 256, H, W)



# revision 13
# speedup vs baseline: 1.0068x; 1.0068x over previous
"""Trainium2 Bass kernel for nn_BasicBlock_1w1a (binary conv BasicBlock).

Self-contained: takes FULL inputs (batch 64), shards batch across 8 NeuronCores,
runs a single SPMD Bass/Tile kernel with in-kernel AllGathers for the
training-mode BatchNorm batch statistics, gathers the full output.

Per block (twice):
  S      = conv3x3(sign(x), sign(w))        # fp8 DoubleRow matmuls, exact
  gate   = sigmoid(BN_dada(avgpool8(x) @ dw))
  u      = prelu(S * alpha * gate, a)       # gate/alpha folded into BN affine
  out    = BN(u) * g + b + x                # batch stats via AllGather

v2 restructure vs baseline:
  - BN statistics via DVE bn_stats (kills the ACT Square pass + accum reads)
  - avgpool stage-1 on GpSimd (was 47us of DVE)
  - mid-block affine as ONE scalar_tensor_tensor; the BN bias B1 is folded
    into the next conv's sign (ACT bias) and the final affine (B1+B2); the
    dada gate is invariant to the per-channel shift (BN inside cancels it)
  - tail affine split across ACT/DVE/GpSimd with immediate per-tile DMA out
  - PE keep-warm matmul chain spanning the exposed stat-AllGather gap
"""
import os
import sys

sys.path.insert(0, "/opt/trn_rl_repo")

import numpy as np
import ml_dtypes

import concourse.bass as bass
import concourse.bacc as bacc
import concourse.tile as tile
import concourse.mybir as mybir
from concourse import bass_utils

P = 128
CI = 2
NIMG = 8
NCORES = 8
H = W = 32
S = H * W
SP = 34 * 34
CH = 2
EPS = 1e-5
MAGIC = 0x5F3759DF
AF = mybir.ActivationFunctionType
ALU = mybir.AluOpType
DT = mybir.dt
X_AXIS = mybir.AxisListType.X

_CACHE = {}


def _build():
    nc = bacc.Bacc("TRN2", target_bir_lowering=False, debug=False,
                   num_devices=NCORES)

    x_in = nc.dram_tensor("x", [NIMG, 256, S], DT.float32, kind="ExternalInput")
    w1_in = nc.dram_tensor("w1sb", [P, CI, 9, 2, P], DT.float8e4,
                           kind="ExternalInput")
    w2_in = nc.dram_tensor("w2sb", [P, CI, 9, 2, P], DT.float8e4,
                           kind="ExternalInput")
    # dada weights split hi/lo bf16: [c_lo, ci, hilo, oi, o_lo]
    dw1_in = nc.dram_tensor("dwt1", [P, CI, 2, 2, P], DT.bfloat16,
                            kind="ExternalInput")
    dw2_in = nc.dram_tensor("dwt2", [P, CI, 2, 2, P], DT.bfloat16,
                            kind="ExternalInput")
    # packed per-channel params: j = 0:alpha 1:a 2:g 3:b 4:dg 5:db -> [P, 6, CI]
    pk1_in = nc.dram_tensor("pk1", [P, 6, CI], DT.float32, kind="ExternalInput")
    pk2_in = nc.dram_tensor("pk2", [P, 6, CI], DT.float32, kind="ExternalInput")
    out_t = nc.dram_tensor("out", [NIMG, 256, S], DT.float32,
                           kind="ExternalOutput")

    with tile.TileContext(nc) as tc:
        with tc.tile_pool(name="big", bufs=1) as big, \
             tc.tile_pool(name="small", bufs=1) as small, \
             tc.tile_pool(name="psum", bufs=3, space="PSUM") as psum_pool, \
             tc.tile_pool(name="psum_y", bufs=2, space="PSUM") as psum_y_pool, \
             tc.tile_pool(name="tmp", bufs=3) as tmppool, \
             tc.tile_pool(name="poola", bufs=2) as poola_pool, \
             tc.tile_pool(name="dram", bufs=1, space="DRAM") as dram:

            # ---- warmup collective: absorbs ncfw init + SPMD launch skew ----
            wu = small.tile([P, 1], DT.float32, tag="wu")
            nc.gpsimd.memset(wu[:], 1.0)
            wu_i = dram.tile([P, 1], DT.float32, tag="wu_i")
            wu_o = dram.tile([P * NCORES, 1], DT.float32, tag="wu_o")
            nc.sync.dma_start(wu_i[:], wu[:])
            nc.gpsimd.collective_compute(
                "AllGather", ALU.bypass, replica_groups=[list(range(NCORES))],
                ins=[wu_i[:].opt()], outs=[wu_o[:].opt()])

            def allreduce_stats(stat_sb, out_sb, widx, name):
                """AllGather [128,4] partials + deterministic local reduce."""
                bi = dram.tile([P, 4], DT.float32, tag=f"bi_{name}{widx}")
                bo = dram.tile([P * NCORES, 4], DT.float32,
                               tag=f"bo_{name}{widx}")
                nc.sync.dma_start(bi[:], stat_sb[:])
                nc.gpsimd.collective_compute(
                    "AllGather", ALU.bypass,
                    replica_groups=[list(range(NCORES))],
                    ins=[bi[:].opt()], outs=[bo[:].opt()])
                gath = small.tile([P, NCORES, 4], DT.float32,
                                  tag=f"gth_{name}{widx}")
                nc.sync.dma_start(
                    gath[:], bo[:].rearrange("(r p) c -> p r c", p=P))
                nc.vector.tensor_reduce(out_sb[:],
                                        gath[:].rearrange("p r c -> p c r"),
                                        axis=X_AXIS, op=ALU.add)

            xt = big.tile([P, NIMG, CI, S], DT.float32, tag="xt")
            ut = big.tile([P, 2, NIMG, S], DT.float32, tag="ut")
            spad = big.tile([P, CI, NIMG, SP], DT.float8e4, tag="spad")
            w1sb = big.tile([P, CI, 9, 2, P], DT.float8e4, tag="w1")
            w2sb = big.tile([P, CI, 9, 2, P], DT.float8e4, tag="w2")
            dwt1 = big.tile([P, CI, 2, 2, P], DT.bfloat16, tag="dwt1")
            dwt2 = big.tile([P, CI, 2, 2, P], DT.bfloat16, tag="dwt2")
            pk1 = big.tile([P, 6, CI], DT.float32, tag="pk1")
            pk2 = big.tile([P, 6, CI], DT.float32, tag="pk2")
            # per-image BN partial stats from bn_stats: [oi, n, 4 groups, (c,m,M2)]
            bnst = {
                1: small.tile([P, 2, NIMG, 12], DT.float32, tag="bnst1",
                              name="bnst1"),
                2: small.tile([P, 2, NIMG, 12], DT.float32, tag="bnst2",
                              name="bnst2"),
            }

            nc.vector.memset(
                spad[:].rearrange("p c n s -> p (c n s)").bitcast(DT.int32), 0)

            # weights/params first so conv1 can start ASAP, then x images
            nc.sync.dma_start(w1sb[:], w1_in[:])
            nc.sync.dma_start(pk1[:], pk1_in[:])
            nc.sync.dma_start(dwt1[:], dw1_in[:])
            for n in range(NIMG):
                xv = x_in[n].rearrange("(ci p) s -> p ci s", p=P)
                for ci in range(CI):
                    nc.sync.dma_start(xt[:, n, ci, :], xv[:, ci, :])
            nc.sync.dma_start(w2sb[:], w2_in[:])
            nc.sync.dma_start(dwt2[:], dw2_in[:])
            nc.sync.dma_start(pk2[:], pk2_in[:])

            def sign_into_spad(n, ci, bias=0.0):
                view = spad[:, ci, n, :].rearrange("p (r c) -> p r c", r=34)
                nc.scalar.activation(
                    view[:, 1:33, 1:33],
                    xt[:, n, ci, :].rearrange("p (h w) -> p h w", h=H),
                    AF.Sign, bias=bias)

            def pools_into(p_t, n, ci):
                # stage 1 (w-dir 8->1) as a 3-op binary tree on GpSimd
                # (keeps the big reads off DVE; GpSimd has no X-reduce)
                t1 = poola_pool.tile([P, 512], DT.float32, tag="poolt1",
                                     name=f"pt1_{n}_{ci}")
                t2 = poola_pool.tile([P, 256], DT.float32, tag="poolt2",
                                     name=f"pt2_{n}_{ci}")
                pa = poola_pool.tile([P, H * 4], DT.float32, tag="poola",
                                     name=f"poola_{n}_{ci}")
                xv = xt[:, n, ci, :].rearrange("p (h pw a b) -> p h pw a b",
                                               h=H, pw=4, a=2)
                nc.gpsimd.tensor_add(
                    t1[:].rearrange("p (h pw b) -> p h pw b", h=H, pw=4),
                    xv[:, :, :, 0, :], xv[:, :, :, 1, :])
                t1v = t1[:].rearrange("p (h pw a b) -> p h pw a b", h=H, pw=4,
                                      a=2)
                nc.gpsimd.tensor_add(
                    t2[:].rearrange("p (h pw b) -> p h pw b", h=H, pw=4),
                    t1v[:, :, :, 0, :], t1v[:, :, :, 1, :])
                t2v = t2[:].rearrange("p (h pw a) -> p h pw a", h=H, pw=4)
                nc.gpsimd.tensor_add(
                    pa[:].rearrange("p (h pw) -> p h pw", h=H),
                    t2v[:, :, :, 0], t2v[:, :, :, 1])
                # stage 2 (h-dir 8->1) on DVE
                nc.vector.tensor_reduce(
                    p_t[:, ci, n, :].rearrange("p (ph pw) -> p ph pw", ph=4),
                    pa[:].rearrange("p (ph hh pw) -> p ph pw hh", ph=4, hh=8),
                    axis=X_AXIS, op=ALU.add)

            def rsqrt_inplace(k, t, e1):
                """k = 1/sqrt(t), all DVE (quake seed + 3 Newton)."""
                ki = k.bitcast(DT.int32)
                nc.vector.tensor_scalar(ki, t.bitcast(DT.int32), 1, None,
                                        ALU.arith_shift_right)
                nc.vector.tensor_scalar(ki, ki, MAGIC, None, ALU.subtract)
                nc.vector.tensor_scalar(ki, ki, -1, None, ALU.mult)
                for _ in range(3):
                    nc.vector.tensor_mul(e1, k, k)
                    nc.vector.tensor_mul(e1, e1, t)
                    nc.vector.tensor_scalar(e1, e1, -0.5, 1.5, ALU.mult,
                                            ALU.add)
                    nc.vector.tensor_mul(k, k, e1)

            p_tiles = {
                1: small.tile([P, CI, NIMG, 16], DT.float32, name="p_t1",
                              tag="p1"),
                2: small.tile([P, CI, NIMG, 16], DT.float32, name="p_t2",
                              tag="p2"),
            }

            def conv_group(widx, wsb, pk, n, oi):
                """9x2 DoubleRow matmuls for one (image, out-half) + evict."""
                sview = spad[:, :, n, :].rearrange("p ci (r c) -> p ci r c",
                                                   r=34)
                ps = psum_pool.tile([P, S], DT.float32, tag="ps",
                                    name=f"ps{widx}_{n}_{oi}")
                for kk in range(9):
                    dy, dx = kk // 3, kk % 3
                    lhsT = wsb[:, :, kk, oi, :]
                    for c2 in range(CH):
                        nc.tensor.matmul(
                            ps[:, c2 * 512:(c2 + 1) * 512], lhsT,
                            sview[:, :, c2 * 16 + dy:c2 * 16 + dy + 16,
                                  dx:dx + 32],
                            start=(kk == 0), stop=(kk == 8),
                            perf_mode=mybir.MatmulPerfMode.DoubleRow)
                # u' = prelu(S, a): gate/alpha fold into BN affine later
                u_sl = ut[:, oi, n, :]
                nc.scalar.activation(u_sl, ps[:], AF.Prelu,
                                     alpha=pk[:, 1, oi:oi + 1])
                # (count, mean, count*var) pairs, one call per 512-chunk
                for c in range(2):
                    nc.vector.bn_stats(
                        bnst[widx][:, oi, n, c * 6:(c + 1) * 6],
                        u_sl[:, c * 512:(c + 1) * 512])

            def dada_block(widx, dwt, pk, gate, ystat, ar_y):
                """pools -> hi/lo -> 16 dada MMs -> BN-dada stats -> AG -> gate."""
                p_t = p_tiles[widx]
                ph = small.tile([P, CI, NIMG * 16], DT.bfloat16, tag=f"ph{widx}")
                pl = small.tile([P, CI, NIMG * 16], DT.bfloat16, tag=f"pl{widx}")
                ysb = small.tile([P, 2, NIMG * 16], DT.float32, tag=f"y{widx}")
                ynst = small.tile([P, 2, 6], DT.float32, tag=f"yn{widx}")
                m_s = small.tile([P, 2, NIMG], DT.float32, tag=f"ms{widx}")
                msq = small.tile([P, 2, 2], DT.float32, tag=f"msq{widx}")

                nc.vector.tensor_copy(ph[:],
                                      p_t[:].rearrange("p c n s -> p c (n s)"))
                nc.vector.tensor_sub(pl[:],
                                     p_t[:].rearrange("p c n s -> p c (n s)"),
                                     ph[:])
                for oi in range(2):
                    psy = psum_y_pool.tile([P, NIMG * 16], DT.float32,
                                           tag="psy", name=f"psy{widx}_{oi}")
                    terms = [(hl, pp) for hl in range(2) for pp in (ph, pl)]
                    for ci in range(CI):
                        for ti, (hl, pp) in enumerate(terms):
                            nc.tensor.matmul(
                                psy[:], dwt[:, ci, hl, oi, :], pp[:, ci, :],
                                start=(ci == 0 and ti == 0),
                                stop=(ci == CI - 1 and ti == len(terms) - 1))
                    nc.scalar.activation(ysb[:, oi, :], psy[:], AF.Copy)
                # per-(o) sums over (n,q)
                for oi in range(2):
                    nc.vector.bn_stats(ynst[:, oi, :], ysb[:, oi, :])
                # per-image means for the sigmoid input
                nc.vector.tensor_reduce(
                    m_s[:], ysb[:].rearrange("p c (n q) -> p c n q", n=NIMG),
                    axis=X_AXIS, op=ALU.add)
                yv = ynst[:].rearrange("p c (g f) -> p c g f", g=2)
                # ysum = 64*(m_e + m_o); ysq = M2_e + M2_o + 64*(m_e^2+m_o^2)
                nc.vector.tensor_reduce(ystat[:, 0:2], yv[:, :, :, 1],
                                        axis=X_AXIS, op=ALU.add)
                nc.vector.tensor_scalar(ystat[:, 0:2], ystat[:, 0:2], 64.0,
                                        None, ALU.mult)
                nc.vector.tensor_mul(msq[:], yv[:, :, :, 1], yv[:, :, :, 1])
                nc.vector.tensor_reduce(ystat[:, 2:4], msq[:], axis=X_AXIS,
                                        op=ALU.add)
                nc.vector.tensor_scalar(ystat[:, 2:4], ystat[:, 2:4], 64.0,
                                        None, ALU.mult)
                m2s = small.tile([P, 2], DT.float32, tag=f"m2s{widx}")
                nc.vector.tensor_reduce(m2s[:], yv[:, :, :, 2], axis=X_AXIS,
                                        op=ALU.add)
                nc.vector.tensor_add(ystat[:, 2:4], ystat[:, 2:4], m2s[:])

                allreduce_stats(ystat, ar_y, widx, "y")

                cnt_y = float(NCORES * NIMG * 16)
                for oi in range(2):
                    t = small.tile([P, 1], DT.float32, tag=f"t{widx}_{oi}")
                    mu = small.tile([P, 1], DT.float32, tag=f"mu{widx}_{oi}")
                    k = small.tile([P, 1], DT.float32, tag=f"k{widx}_{oi}")
                    e1 = small.tile([P, 1], DT.float32, tag=f"e{widx}_{oi}")
                    A16 = small.tile([P, 1], DT.float32, tag=f"A{widx}_{oi}")
                    B = small.tile([P, 1], DT.float32, tag=f"B{widx}_{oi}")
                    nc.vector.tensor_scalar(t[:], ar_y[:, 2 + oi:3 + oi],
                                            1.0 / cnt_y, EPS, ALU.mult, ALU.add)
                    nc.vector.tensor_scalar(mu[:], ar_y[:, oi:oi + 1],
                                            1.0 / cnt_y, None, ALU.mult)
                    nc.vector.tensor_mul(e1[:], mu[:], mu[:])
                    nc.vector.tensor_sub(t[:], t[:], e1[:])
                    rsqrt_inplace(k[:], t[:], e1[:])
                    # A = k*dg; sigmoid(A*(m_s/16) + B): scale = A/16
                    nc.vector.tensor_mul(A16[:], k[:], pk[:, 4, oi:oi + 1])
                    nc.vector.tensor_mul(B[:], mu[:], A16[:])
                    nc.vector.tensor_sub(B[:], pk[:, 5, oi:oi + 1], B[:])
                    nc.vector.tensor_scalar(A16[:], A16[:], 1.0 / 16.0, None,
                                            ALU.mult)
                    sig = small.tile([P, NIMG], DT.float32,
                                     tag=f"sg{widx}_{oi}")
                    nc.scalar.activation(sig[:], m_s[:, oi, :], AF.Sigmoid,
                                         bias=B[:], scale=A16[:])
                    nc.vector.tensor_scalar(gate[:, oi, :], sig[:],
                                            pk[:, 0, oi:oi + 1], None, ALU.mult)

            def main_stats(widx, gate, ustat):
                """usum/usq per image from bn_stats partials, gate-weighted."""
                bv = bnst[widx][:].rearrange("p c n (g f) -> p c n g f", g=4)
                ms = small.tile([P, 2, NIMG], DT.float32, tag=f"us_m{widx}")
                mq = small.tile([P, 2, NIMG, 4], DT.float32, tag=f"us_q{widx}")
                qs = small.tile([P, 2, NIMG], DT.float32, tag=f"us_s{widx}")
                m2 = small.tile([P, 2, NIMG], DT.float32, tag=f"us_2{widx}")
                w8 = small.tile([P, 2, NIMG], DT.float32, tag=f"us_w{widx}")
                g2 = small.tile([P, 2, NIMG], DT.float32, tag=f"us_g{widx}")
                # sum(u) per (oi,n) = 256 * sum of 4 group means
                nc.vector.tensor_reduce(ms[:], bv[:, :, :, :, 1], axis=X_AXIS,
                                        op=ALU.add)
                # sum(u^2) = sum M2 + 256 * sum m^2
                nc.vector.tensor_mul(mq[:], bv[:, :, :, :, 1], bv[:, :, :, :, 1])
                nc.vector.tensor_reduce(qs[:], mq[:], axis=X_AXIS, op=ALU.add)
                nc.vector.tensor_reduce(m2[:], bv[:, :, :, :, 2], axis=X_AXIS,
                                        op=ALU.add)
                nc.vector.tensor_scalar(qs[:], qs[:], 256.0, None, ALU.mult)
                nc.vector.tensor_add(qs[:], qs[:], m2[:])
                # gate-weighted: sum_n g*usum, sum_n g^2*usq   (256 into scale)
                nc.vector.tensor_mul(w8[:], ms[:], gate[:])
                nc.vector.tensor_reduce(ustat[:, 0:2], w8[:], axis=X_AXIS,
                                        op=ALU.add)
                nc.vector.tensor_scalar(ustat[:, 0:2], ustat[:, 0:2], 256.0,
                                        None, ALU.mult)
                nc.vector.tensor_mul(g2[:], gate[:], gate[:])
                nc.vector.tensor_mul(w8[:], qs[:], g2[:])
                nc.vector.tensor_reduce(ustat[:, 2:4], w8[:], axis=X_AXIS,
                                        op=ALU.add)

            def bn_affine(widx, pk, ar_u, gate, AB, gA):
                """A = k*g, B = b - A*mu, gA[n] = A*gate[n]."""
                cnt_u = float(NCORES * NIMG * S)
                for ci in range(2):
                    t = small.tile([P, 1], DT.float32, tag=f"tu{widx}_{ci}")
                    mu = small.tile([P, 1], DT.float32, tag=f"muu{widx}_{ci}")
                    k = small.tile([P, 1], DT.float32, tag=f"ku{widx}_{ci}")
                    e1 = small.tile([P, 1], DT.float32, tag=f"eu{widx}_{ci}")
                    nc.vector.tensor_scalar(t[:], ar_u[:, 2 + ci:3 + ci],
                                            1.0 / cnt_u, EPS, ALU.mult, ALU.add)
                    nc.vector.tensor_scalar(mu[:], ar_u[:, ci:ci + 1],
                                            1.0 / cnt_u, None, ALU.mult)
                    nc.vector.tensor_mul(e1[:], mu[:], mu[:])
                    nc.vector.tensor_sub(t[:], t[:], e1[:])
                    rsqrt_inplace(k[:], t[:], e1[:])
                    nc.vector.tensor_mul(AB[:, 0, ci:ci + 1], k[:],
                                         pk[:, 2, ci:ci + 1])
                    nc.vector.tensor_mul(e1[:], mu[:], AB[:, 0, ci:ci + 1])
                    nc.vector.tensor_sub(AB[:, 1, ci:ci + 1],
                                         pk[:, 3, ci:ci + 1], e1[:])
                for ci in range(2):
                    nc.vector.tensor_scalar(gA[:, ci, :], gate[:, ci, :],
                                            AB[:, 0, ci:ci + 1], None, ALU.mult)

            gate1 = small.tile([P, 2, NIMG], DT.float32, tag="g1")
            gate2 = small.tile([P, 2, NIMG], DT.float32, tag="g2")
            ystat1 = small.tile([P, 4], DT.float32, tag="ys1")
            ystat2 = small.tile([P, 4], DT.float32, tag="ys2")
            ar_y1 = small.tile([P, 4], DT.float32, tag="ary1")
            ar_y2 = small.tile([P, 4], DT.float32, tag="ary2")
            ustat1 = small.tile([P, 4], DT.float32, tag="us1")
            ustat2 = small.tile([P, 4], DT.float32, tag="us2")
            ar_u1 = small.tile([P, 4], DT.float32, tag="aru1")
            ar_u2 = small.tile([P, 4], DT.float32, tag="aru2")
            AB1 = small.tile([P, 2, 2], DT.float32, tag="ab1")
            AB2 = small.tile([P, 2, 2], DT.float32, tag="ab2")
            gA1 = small.tile([P, 2, NIMG], DT.float32, tag="ga1")
            gA2 = small.tile([P, 2, NIMG], DT.float32, tag="ga2")
            Bp = small.tile([P, 2], DT.float32, tag="bp")

            # ================= block 1 =================
            for n in range(NIMG):
                for ci in range(CI):
                    sign_into_spad(n, ci)
                    pools_into(p_tiles[1], n, ci)

            # conv1 matmuls (image-major); dada1 emitted after image 1
            for n in range(NIMG):
                for oi in range(2):
                    conv_group(1, w1sb, pk1, n, oi)
                if n == 1:
                    dada_block(1, dwt1, pk1, gate1, ystat1, ar_y1)

            main_stats(1, gate1, ustat1)
            allreduce_stats(ustat1, ar_u1, 1, "u")

            # keep-warm chain: fp32 MMs reading ut (ready at conv1 end),
            # spans the AllGather gap so conv2 starts at K=8/8
            pd = psum_pool.tile([P, 512], DT.float32, tag="ps", name="pd_warm")
            for i in range(5):
                nc.tensor.matmul(pd[:], ut[:, 0, 0, 0:P], ut[:, 1, 7, 0:512],
                                 start=(i == 0), stop=(i == 4))

            bn_affine(1, pk1, ar_u1, gate1, AB1, gA1)

            # x1 = gA1[n]*u' + x (B1 folded into sign bias / final affine;
            # the dada gate is invariant to the per-channel shift)
            for n in range(NIMG):
                for ci in range(CI):
                    nc.vector.scalar_tensor_tensor(
                        xt[:, n, ci, :], ut[:, ci, n, :],
                        gA1[:, ci, n:n + 1], xt[:, n, ci, :],
                        ALU.mult, ALU.add)
                    sign_into_spad(n, ci, bias=AB1[:, 1, ci:ci + 1])
                    pools_into(p_tiles[2], n, ci)

            # ================= block 2 =================
            for n in range(NIMG):
                for oi in range(2):
                    conv_group(2, w2sb, pk2, n, oi)
                if n == 1:
                    dada_block(2, dwt2, pk2, gate2, ystat2, ar_y2)

            main_stats(2, gate2, ustat2)
            allreduce_stats(ustat2, ar_u2, 2, "u")
            bn_affine(2, pk2, ar_u2, gate2, AB2, gA2)
            # B' = B1 + B2 (skip path carries the un-shifted v = x1 - B1)
            nc.vector.tensor_add(Bp[:], AB1[:, 1, :], AB2[:, 1, :])

            # out = gA2[n]*u' + B' + v ; 3-engine split + immediate DMA
            for n in range(NIMG):
                for ci in range(CI):
                    idx = n * 2 + ci
                    ov = out_t[n].rearrange("(ci p) s -> p ci s", p=P)
                    tmp = tmppool.tile([P, S], DT.float32, tag="tmp",
                                       name=f"tmp2_{n}_{ci}")
                    if idx % 8 < 5:
                        nc.scalar.activation(tmp[:], ut[:, ci, n, :],
                                             AF.Identity,
                                             bias=Bp[:, ci:ci + 1],
                                             scale=gA2[:, ci, n:n + 1])
                    else:
                        nc.vector.tensor_scalar(tmp[:], ut[:, ci, n, :],
                                                gA2[:, ci, n:n + 1],
                                                Bp[:, ci:ci + 1],
                                                ALU.mult, ALU.add)
                    eng = nc.gpsimd if idx % 2 == 0 else nc.vector
                    eng.tensor_add(xt[:, n, ci, :], tmp[:], xt[:, n, ci, :])
                    nc.sync.dma_start(ov[:, ci, :], xt[:, n, ci, :])

    nc.compile()
    return nc


def _pack_w(w):
    ws = np.sign(w.astype(np.float32))
    t = ws.reshape(2, P, CI, P, 3, 3)           # oi, o_lo, ci, c_lo, dy, dx
    t = t.transpose(3, 2, 4, 5, 0, 1)           # c_lo, ci, dy, dx, oi, o_lo
    return np.ascontiguousarray(t.reshape(P, CI, 9, 2, P)).astype(
        ml_dtypes.float8_e4m3)


def _pack_dw(dw):
    d = (dw.astype(np.float32) / 64.0).reshape(2, P, CI, P)  # oi,o_lo,ci,c_lo
    d = d.transpose(3, 2, 0, 1)                               # c_lo,ci,oi,o_lo
    hi = d.astype(ml_dtypes.bfloat16)
    lo = (d - hi.astype(np.float32)).astype(ml_dtypes.bfloat16)
    out = np.empty((P, CI, 2, 2, P), ml_dtypes.bfloat16)
    out[:, :, 0] = hi
    out[:, :, 1] = lo
    return out


def _pack_pk(w, a, g, b, dg, db):
    alpha = np.abs(w.astype(np.float32)).mean(axis=(1, 2, 3))
    fields = [alpha, a, g, b, dg, db]
    pk = np.empty((P, 6, CI), np.float32)
    for j, f in enumerate(fields):
        pk[:, j, :] = np.asarray(f, np.float32).reshape(CI, P).T
    return pk


def kernel(**inputs):
    if "nc" not in _CACHE:
        _CACHE["nc"] = _build()
    nc = _CACHE["nc"]

    x = np.asarray(inputs["x"], np.float32).reshape(64, 256, S)
    feed = {
        "w1sb": _pack_w(np.asarray(inputs["w1"])),
        "w2sb": _pack_w(np.asarray(inputs["w2"])),
        "dwt1": _pack_dw(np.asarray(inputs["dw1"])),
        "dwt2": _pack_dw(np.asarray(inputs["dw2"])),
        "pk1": _pack_pk(np.asarray(inputs["w1"]), inputs["a1"], inputs["g1"],
                        inputs["b1"], inputs["dg1"], inputs["db1"]),
        "pk2": _pack_pk(np.asarray(inputs["w2"]), inputs["a2"], inputs["g2"],
                        inputs["b2"], inputs["dg2"], inputs["db2"]),
    }
    in_maps = []
    for c in range(NCORES):
        m = dict(feed)
        m["x"] = np.ascontiguousarray(x[c * NIMG:(c + 1) * NIMG])
        in_maps.append(m)

    trace = bool(int(os.environ.get("BASS_KERNEL_TRACE", "0")))
    res = bass_utils.run_bass_kernel_spmd(
        nc, in_maps, core_ids=list(range(NCORES)), trace=trace)
    kernel.last_results = res

    out = np.concatenate([res.results[c]["out"] for c in range(NCORES)], axis=0)
    return out.reshape(64, 256, H, W)


# revision 15
# speedup vs baseline: 1.0737x; 1.0664x over previous
"""Trainium2 Bass kernel for nn_BasicBlock_1w1a (binary conv BasicBlock).

Self-contained: takes FULL inputs (batch 64), shards batch across 8 NeuronCores,
runs a single SPMD Bass/Tile kernel with in-kernel AllGathers for the
training-mode BatchNorm batch statistics, gathers the full output.

Per block (twice):
  S      = conv3x3(sign(x), sign(w))        # fp8 DoubleRow matmuls, exact
  gate   = sigmoid(BN_dada(avgpool8(x) @ dw))
  u      = prelu(S * alpha * gate, a)       # gate/alpha folded into BN affine
  out    = BN(u) * g + b + x                # batch stats via AllGather

v3 structure:
  - conv matmuls grouped (oi, img-quad, row-half): one LDWEIGHTS feeds 4
    N=512 matmuls; 8x [128,512] psum ring
  - BN statistics via DVE bn_stats on each 512-px eviction (no Square pass)
  - block-2 dada pools from pool(u') and pool(x): p2 = gA1*pool_u + p1
    (the per-channel +B1 shift cancels inside the dada BN), so the dada2
    matmuls/AllGather run inside the stat-AllGather gap / conv2 window
  - mid-block affine: DVE scale + GpSimd add; B1 folded into the next
    sign's ACT bias and the final affine's B1+B2
  - avgpool stage-1 as GpSimd add-trees, stage-2 DVE
  - tail affine split ACT/DVE/GpSimd with immediate per-tile DMA out
  - PE keep-warm fp32 matmul chain spanning the exposed AllGather gap
"""
import os
import sys

sys.path.insert(0, "/opt/trn_rl_repo")

import numpy as np
import ml_dtypes

import concourse.bass as bass
import concourse.bacc as bacc
import concourse.tile as tile
import concourse.mybir as mybir
from concourse import bass_utils

P = 128
CI = 2
NIMG = 8
NCORES = 8
H = W = 32
S = H * W
SP = 34 * 34
EPS = 1e-5
MAGIC = 0x5F3759DF
AF = mybir.ActivationFunctionType
ALU = mybir.AluOpType
DT = mybir.dt
X_AXIS = mybir.AxisListType.X

_CACHE = {}


def _build():
    nc = bacc.Bacc("TRN2", target_bir_lowering=False, debug=False,
                   num_devices=NCORES)

    x_in = nc.dram_tensor("x", [NIMG, 256, S], DT.float32, kind="ExternalInput")
    w1_in = nc.dram_tensor("w1sb", [P, CI, 9, 2, P], DT.float8e4,
                           kind="ExternalInput")
    w2_in = nc.dram_tensor("w2sb", [P, CI, 9, 2, P], DT.float8e4,
                           kind="ExternalInput")
    # dada weights split hi/lo bf16: [c_lo, ci, hilo, oi, o_lo]
    dw1_in = nc.dram_tensor("dwt1", [P, CI, 2, 2, P], DT.bfloat16,
                            kind="ExternalInput")
    dw2_in = nc.dram_tensor("dwt2", [P, CI, 2, 2, P], DT.bfloat16,
                            kind="ExternalInput")
    # packed per-channel params: j = 0:alpha 1:a 2:g 3:b 4:dg 5:db -> [P, 6, CI]
    pk1_in = nc.dram_tensor("pk1", [P, 6, CI], DT.float32, kind="ExternalInput")
    pk2_in = nc.dram_tensor("pk2", [P, 6, CI], DT.float32, kind="ExternalInput")
    out_t = nc.dram_tensor("out", [NIMG, 256, S], DT.float32,
                           kind="ExternalOutput")

    with tile.TileContext(nc) as tc:
        with tc.tile_pool(name="big", bufs=1) as big, \
             tc.tile_pool(name="small", bufs=1) as small, \
             tc.tile_pool(name="psum", bufs=8, space="PSUM") as psum_pool, \
             tc.tile_pool(name="tmp", bufs=4) as tmppool, \
             tc.tile_pool(name="poola", bufs=3) as poola_pool, \
             tc.tile_pool(name="dram", bufs=1, space="DRAM") as dram:

            # ---- warmup collective: absorbs ncfw init + SPMD launch skew ----
            wu = small.tile([P, 1], DT.float32, tag="wu")
            nc.gpsimd.memset(wu[:], 1.0)
            wu_i = dram.tile([P, 1], DT.float32, tag="wu_i")
            wu_o = dram.tile([P * NCORES, 1], DT.float32, tag="wu_o")
            nc.sync.dma_start(wu_i[:], wu[:])
            nc.gpsimd.collective_compute(
                "AllGather", ALU.bypass, replica_groups=[list(range(NCORES))],
                ins=[wu_i[:].opt()], outs=[wu_o[:].opt()])

            def allreduce_stats(stat_sb, out_sb, widx, name):
                """AllGather [128,4] partials + deterministic local reduce."""
                bi = dram.tile([P, 4], DT.float32, tag=f"bi_{name}{widx}")
                bo = dram.tile([P * NCORES, 4], DT.float32,
                               tag=f"bo_{name}{widx}")
                nc.sync.dma_start(bi[:], stat_sb[:])
                nc.gpsimd.collective_compute(
                    "AllGather", ALU.bypass,
                    replica_groups=[list(range(NCORES))],
                    ins=[bi[:].opt()], outs=[bo[:].opt()])
                gath = small.tile([P, NCORES, 4], DT.float32,
                                  tag=f"gth_{name}{widx}")
                nc.sync.dma_start(
                    gath[:], bo[:].rearrange("(r p) c -> p r c", p=P))
                nc.vector.tensor_reduce(out_sb[:],
                                        gath[:].rearrange("p r c -> p c r"),
                                        axis=X_AXIS, op=ALU.add)

            xt = big.tile([P, NIMG, CI, S], DT.float32, tag="xt")
            ut = big.tile([P, 2, NIMG, S], DT.float32, tag="ut")
            spad = big.tile([P, CI, NIMG, SP], DT.float8e4, tag="spad")
            w1sb = big.tile([P, CI, 9, 2, P], DT.float8e4, tag="w1")
            w2sb = big.tile([P, CI, 9, 2, P], DT.float8e4, tag="w2")
            dwt1 = big.tile([P, CI, 2, 2, P], DT.bfloat16, tag="dwt1")
            dwt2 = big.tile([P, CI, 2, 2, P], DT.bfloat16, tag="dwt2")
            pk1 = big.tile([P, 6, CI], DT.float32, tag="pk1")
            pk2 = big.tile([P, 6, CI], DT.float32, tag="pk2")
            # per-(img,half) BN partials from bn_stats: [oi, n, 4 grp, (c,m,M2)]
            bnst = {
                1: small.tile([P, 2, NIMG, 12], DT.float32, tag="bnst1",
                              name="bnst1"),
                2: small.tile([P, 2, NIMG, 12], DT.float32, tag="bnst2",
                              name="bnst2"),
            }
            # pool sums of u' (block1) for the dada2-pools identity
            pools_u = big.tile([P, 2, NIMG, 16], DT.float32, tag="pu")

            nc.vector.memset(
                spad[:].rearrange("p c n s -> p (c n s)").bitcast(DT.int32), 0)

            def dma_x(n):
                xv = x_in[n].rearrange("(ci p) s -> p ci s", p=P)
                for ci in range(CI):
                    nc.sync.dma_start(xt[:, n, ci, :], xv[:, ci, :])

            for n in range(4):
                dma_x(n)
            nc.sync.dma_start(w1sb[:], w1_in[:])
            nc.sync.dma_start(pk1[:], pk1_in[:])
            nc.sync.dma_start(dwt1[:], dw1_in[:])
            for n in range(4, NIMG):
                dma_x(n)
            nc.sync.dma_start(w2sb[:], w2_in[:])
            nc.sync.dma_start(dwt2[:], dw2_in[:])
            nc.sync.dma_start(pk2[:], pk2_in[:])

            def sign_into_spad(n, ci, bias=0.0):
                view = spad[:, ci, n, :].rearrange("p (r c) -> p r c", r=34)
                nc.scalar.activation(
                    view[:, 1:33, 1:33],
                    xt[:, n, ci, :].rearrange("p (h w) -> p h w", h=H),
                    AF.Sign, bias=bias)

            def pool_tree(src_ap, dst_16, key):
                """8x8 sum-pool of one [P, 1024] (h,w) image plane.

                stage 1 (w-dir) = 3 GpSimd add-tree ops; stage 2 (h) on DVE.
                """
                t1 = poola_pool.tile([P, 512], DT.float32, tag="poolt1",
                                     name=f"pt1_{key}")
                t2 = poola_pool.tile([P, 256], DT.float32, tag="poolt2",
                                     name=f"pt2_{key}")
                pa = poola_pool.tile([P, H * 4], DT.float32, tag="poola",
                                     name=f"pa_{key}")
                xv = src_ap.rearrange("p (h pw a b) -> p h pw a b", h=H,
                                      pw=4, a=2)
                nc.gpsimd.tensor_add(
                    t1[:].rearrange("p (h pw b) -> p h pw b", h=H, pw=4),
                    xv[:, :, :, 0, :], xv[:, :, :, 1, :])
                t1v = t1[:].rearrange("p (h pw a b) -> p h pw a b", h=H,
                                      pw=4, a=2)
                nc.gpsimd.tensor_add(
                    t2[:].rearrange("p (h pw b) -> p h pw b", h=H, pw=4),
                    t1v[:, :, :, 0, :], t1v[:, :, :, 1, :])
                t2v = t2[:].rearrange("p (h pw a) -> p h pw a", h=H, pw=4)
                nc.gpsimd.tensor_add(
                    pa[:].rearrange("p (h pw) -> p h pw", h=H),
                    t2v[:, :, :, 0], t2v[:, :, :, 1])
                nc.vector.tensor_reduce(
                    dst_16.rearrange("p (ph pw) -> p ph pw", ph=4),
                    pa[:].rearrange("p (ph hh pw) -> p ph pw hh", ph=4, hh=8),
                    axis=X_AXIS, op=ALU.add)

            def rsqrt_inplace(k, t, e1):
                """k = 1/sqrt(t), all DVE (quake seed + 3 Newton)."""
                ki = k.bitcast(DT.int32)
                nc.vector.tensor_scalar(ki, t.bitcast(DT.int32), 1, None,
                                        ALU.arith_shift_right)
                nc.vector.tensor_scalar(ki, ki, MAGIC, None, ALU.subtract)
                nc.vector.tensor_scalar(ki, ki, -1, None, ALU.mult)
                for _ in range(3):
                    nc.vector.tensor_mul(e1, k, k)
                    nc.vector.tensor_mul(e1, e1, t)
                    nc.vector.tensor_scalar(e1, e1, -0.5, 1.5, ALU.mult,
                                            ALU.add)
                    nc.vector.tensor_mul(k, k, e1)

            p_tiles = {
                1: small.tile([P, CI, NIMG, 16], DT.float32, name="p_t1",
                              tag="p1"),
                2: small.tile([P, CI, NIMG, 16], DT.float32, name="p_t2",
                              tag="p2"),
            }

            def conv_quad(widx, wsb, pk, oi, imgs, half):
                """one LDW per kk feeds len(imgs) N=512 DoubleRow matmuls."""
                tl = {n: psum_pool.tile([P, 512], DT.float32, tag="ps",
                                        name=f"ps{widx}_{oi}_{half}_{n}")
                      for n in imgs}
                for kk in range(9):
                    dy, dx = divmod(kk, 3)
                    lhsT = wsb[:, :, kk, oi, :]
                    for j, n in enumerate(imgs):
                        sview = spad[:, :, n, :].rearrange(
                            "p ci (r c) -> p ci r c", r=34)
                        mm = nc.tensor.matmul(
                            tl[n][:], lhsT,
                            sview[:, :, half * 16 + dy:half * 16 + dy + 16,
                                  dx:dx + 32],
                            start=(kk == 0), stop=(kk == 8),
                            perf_mode=mybir.MatmulPerfMode.DoubleRow)
                        if j > 0:
                            # same stationary weights as the j==0 matmul of
                            # this kk — skip the redundant LDWEIGHTS
                            mm.ins.ldweights = False
                for n in imgs:
                    u_sl = ut[:, oi, n, half * 512:(half + 1) * 512]
                    nc.scalar.activation(u_sl, tl[n][:], AF.Prelu,
                                         alpha=pk[:, 1, oi:oi + 1])
                    nc.vector.bn_stats(
                        bnst[widx][:, oi, n, half * 6:(half + 1) * 6], u_sl)
                    if widx == 1 and half == 1:
                        pool_tree(ut[:, oi, n, :], pools_u[:, oi, n, :],
                                  f"u_{oi}_{n}")

            def dada_block(widx, dwt, pk, p_t, gate, ystat, ar_y):
                """hi/lo -> 16 dada MMs -> BN-dada stats -> AG -> gate."""
                ph = small.tile([P, CI, NIMG * 16], DT.bfloat16, tag=f"ph{widx}")
                pl = small.tile([P, CI, NIMG * 16], DT.bfloat16, tag=f"pl{widx}")
                ysb = small.tile([P, 2, NIMG * 16], DT.float32, tag=f"y{widx}")
                ynst = small.tile([P, 2, 6], DT.float32, tag=f"yn{widx}")
                m_s = small.tile([P, 2, NIMG], DT.float32, tag=f"ms{widx}")
                msq = small.tile([P, 2, 2], DT.float32, tag=f"msq{widx}")

                nc.vector.tensor_copy(ph[:],
                                      p_t[:].rearrange("p c n s -> p c (n s)"))
                nc.vector.tensor_sub(pl[:],
                                     p_t[:].rearrange("p c n s -> p c (n s)"),
                                     ph[:])
                for oi in range(2):
                    psy = psum_pool.tile([P, NIMG * 16], DT.float32,
                                         tag="ps", name=f"psy{widx}_{oi}")
                    terms = [(hl, pp) for hl in range(2) for pp in (ph, pl)]
                    for ci in range(CI):
                        for ti, (hl, pp) in enumerate(terms):
                            nc.tensor.matmul(
                                psy[:], dwt[:, ci, hl, oi, :], pp[:, ci, :],
                                start=(ci == 0 and ti == 0),
                                stop=(ci == CI - 1 and ti == len(terms) - 1))
                    nc.scalar.activation(ysb[:, oi, :], psy[:], AF.Copy)
                for oi in range(2):
                    nc.vector.bn_stats(ynst[:, oi, :], ysb[:, oi, :])
                nc.vector.tensor_reduce(
                    m_s[:], ysb[:].rearrange("p c (n q) -> p c n q", n=NIMG),
                    axis=X_AXIS, op=ALU.add)
                yv = ynst[:].rearrange("p c (g f) -> p c g f", g=2)
                # ysum = 64*(m_e + m_o); ysq = M2_e + M2_o + 64*(m_e^2+m_o^2)
                nc.vector.tensor_reduce(ystat[:, 0:2], yv[:, :, :, 1],
                                        axis=X_AXIS, op=ALU.add)
                nc.vector.tensor_scalar(ystat[:, 0:2], ystat[:, 0:2], 64.0,
                                        None, ALU.mult)
                nc.vector.tensor_mul(msq[:], yv[:, :, :, 1], yv[:, :, :, 1])
                nc.vector.tensor_reduce(ystat[:, 2:4], msq[:], axis=X_AXIS,
                                        op=ALU.add)
                nc.vector.tensor_scalar(ystat[:, 2:4], ystat[:, 2:4], 64.0,
                                        None, ALU.mult)
                m2s = small.tile([P, 2], DT.float32, tag=f"m2s{widx}")
                nc.vector.tensor_reduce(m2s[:], yv[:, :, :, 2], axis=X_AXIS,
                                        op=ALU.add)
                nc.vector.tensor_add(ystat[:, 2:4], ystat[:, 2:4], m2s[:])

                allreduce_stats(ystat, ar_y, widx, "y")

                cnt_y = float(NCORES * NIMG * 16)
                for oi in range(2):
                    t = small.tile([P, 1], DT.float32, tag=f"t{widx}_{oi}")
                    mu = small.tile([P, 1], DT.float32, tag=f"mu{widx}_{oi}")
                    k = small.tile([P, 1], DT.float32, tag=f"k{widx}_{oi}")
                    e1 = small.tile([P, 1], DT.float32, tag=f"e{widx}_{oi}")
                    A16 = small.tile([P, 1], DT.float32, tag=f"A{widx}_{oi}")
                    B = small.tile([P, 1], DT.float32, tag=f"B{widx}_{oi}")
                    nc.vector.tensor_scalar(t[:], ar_y[:, 2 + oi:3 + oi],
                                            1.0 / cnt_y, EPS, ALU.mult, ALU.add)
                    nc.vector.tensor_scalar(mu[:], ar_y[:, oi:oi + 1],
                                            1.0 / cnt_y, None, ALU.mult)
                    nc.vector.tensor_mul(e1[:], mu[:], mu[:])
                    nc.vector.tensor_sub(t[:], t[:], e1[:])
                    rsqrt_inplace(k[:], t[:], e1[:])
                    # A = k*dg; sigmoid(A*(m_s/16) + B): scale = A/16
                    nc.vector.tensor_mul(A16[:], k[:], pk[:, 4, oi:oi + 1])
                    nc.vector.tensor_mul(B[:], mu[:], A16[:])
                    nc.vector.tensor_sub(B[:], pk[:, 5, oi:oi + 1], B[:])
                    nc.vector.tensor_scalar(A16[:], A16[:], 1.0 / 16.0, None,
                                            ALU.mult)
                    sig = small.tile([P, NIMG], DT.float32,
                                     tag=f"sg{widx}_{oi}")
                    nc.scalar.activation(sig[:], m_s[:, oi, :], AF.Sigmoid,
                                         bias=B[:], scale=A16[:])
                    nc.vector.tensor_scalar(gate[:, oi, :], sig[:],
                                            pk[:, 0, oi:oi + 1], None, ALU.mult)

            def main_stats(widx, gate, ustat):
                """usum/usq per image from bn_stats partials, gate-weighted."""
                bv = bnst[widx][:].rearrange("p c n (g f) -> p c n g f", g=4)
                ms = small.tile([P, 2, NIMG], DT.float32, tag=f"us_m{widx}")
                mq = small.tile([P, 2, NIMG, 4], DT.float32, tag=f"us_q{widx}")
                qs = small.tile([P, 2, NIMG], DT.float32, tag=f"us_s{widx}")
                m2 = small.tile([P, 2, NIMG], DT.float32, tag=f"us_2{widx}")
                w8 = small.tile([P, 2, NIMG], DT.float32, tag=f"us_w{widx}")
                g2 = small.tile([P, 2, NIMG], DT.float32, tag=f"us_g{widx}")
                # sum(u) per (oi,n) = 256 * sum of 4 group means
                nc.vector.tensor_reduce(ms[:], bv[:, :, :, :, 1], axis=X_AXIS,
                                        op=ALU.add)
                # sum(u^2) = sum M2 + 256 * sum m^2
                nc.vector.tensor_mul(mq[:], bv[:, :, :, :, 1],
                                     bv[:, :, :, :, 1])
                nc.vector.tensor_reduce(qs[:], mq[:], axis=X_AXIS, op=ALU.add)
                nc.vector.tensor_reduce(m2[:], bv[:, :, :, :, 2], axis=X_AXIS,
                                        op=ALU.add)
                nc.vector.tensor_scalar(qs[:], qs[:], 256.0, None, ALU.mult)
                nc.vector.tensor_add(qs[:], qs[:], m2[:])
                # gate-weighted: sum_n g*usum, sum_n g^2*usq   (256 into scale)
                nc.vector.tensor_mul(w8[:], ms[:], gate[:])
                nc.vector.tensor_reduce(ustat[:, 0:2], w8[:], axis=X_AXIS,
                                        op=ALU.add)
                nc.vector.tensor_scalar(ustat[:, 0:2], ustat[:, 0:2], 256.0,
                                        None, ALU.mult)
                nc.vector.tensor_mul(g2[:], gate[:], gate[:])
                nc.vector.tensor_mul(w8[:], qs[:], g2[:])
                nc.vector.tensor_reduce(ustat[:, 2:4], w8[:], axis=X_AXIS,
                                        op=ALU.add)

            def bn_affine(widx, pk, ar_u, gate, AB, gA):
                """A = k*g, B = b - A*mu, gA[n] = A*gate[n]."""
                cnt_u = float(NCORES * NIMG * S)
                for ci in range(2):
                    t = small.tile([P, 1], DT.float32, tag=f"tu{widx}_{ci}")
                    mu = small.tile([P, 1], DT.float32, tag=f"muu{widx}_{ci}")
                    k = small.tile([P, 1], DT.float32, tag=f"ku{widx}_{ci}")
                    e1 = small.tile([P, 1], DT.float32, tag=f"eu{widx}_{ci}")
                    nc.vector.tensor_scalar(t[:], ar_u[:, 2 + ci:3 + ci],
                                            1.0 / cnt_u, EPS, ALU.mult, ALU.add)
                    nc.vector.tensor_scalar(mu[:], ar_u[:, ci:ci + 1],
                                            1.0 / cnt_u, None, ALU.mult)
                    nc.vector.tensor_mul(e1[:], mu[:], mu[:])
                    nc.vector.tensor_sub(t[:], t[:], e1[:])
                    rsqrt_inplace(k[:], t[:], e1[:])
                    nc.vector.tensor_mul(AB[:, 0, ci:ci + 1], k[:],
                                         pk[:, 2, ci:ci + 1])
                    nc.vector.tensor_mul(e1[:], mu[:], AB[:, 0, ci:ci + 1])
                    nc.vector.tensor_sub(AB[:, 1, ci:ci + 1],
                                         pk[:, 3, ci:ci + 1], e1[:])
                for ci in range(2):
                    nc.vector.tensor_scalar(gA[:, ci, :], gate[:, ci, :],
                                            AB[:, 0, ci:ci + 1], None, ALU.mult)

            gate1 = small.tile([P, 2, NIMG], DT.float32, tag="g1")
            gate2 = small.tile([P, 2, NIMG], DT.float32, tag="g2")
            ystat1 = small.tile([P, 4], DT.float32, tag="ys1")
            ystat2 = small.tile([P, 4], DT.float32, tag="ys2")
            ar_y1 = small.tile([P, 4], DT.float32, tag="ary1")
            ar_y2 = small.tile([P, 4], DT.float32, tag="ary2")
            ustat1 = small.tile([P, 4], DT.float32, tag="us1")
            ustat2 = small.tile([P, 4], DT.float32, tag="us2")
            ar_u1 = small.tile([P, 4], DT.float32, tag="aru1")
            ar_u2 = small.tile([P, 4], DT.float32, tag="aru2")
            AB1 = small.tile([P, 2, 2], DT.float32, tag="ab1")
            AB2 = small.tile([P, 2, 2], DT.float32, tag="ab2")
            gA1 = small.tile([P, 2, NIMG], DT.float32, tag="ga1")
            gA2 = small.tile([P, 2, NIMG], DT.float32, tag="ga2")
            Bp = small.tile([P, 2], DT.float32, tag="bp")

            QUADS = [(0, 1, 2, 3), (4, 5, 6, 7)]

            # ================= block 1 =================
            for n in range(NIMG):
                for ci in range(CI):
                    sign_into_spad(n, ci)
                    pool_tree(xt[:, n, ci, :], p_tiles[1][:, ci, n, :],
                              f"x1_{n}_{ci}")

            for oi in range(2):
                for imgs in QUADS:
                    for half in range(2):
                        conv_quad(1, w1sb, pk1, oi, imgs, half)
                if oi == 0:
                    dada_block(1, dwt1, pk1, p_tiles[1], gate1, ystat1, ar_y1)

            main_stats(1, gate1, ustat1)
            allreduce_stats(ustat1, ar_u1, 1, "u")

            # keep-warm chain: fp32 MMs reading ut (ready at conv1 end),
            # spans the AllGather gap so conv2 starts at K=8/8
            pd = psum_pool.tile([P, 512], DT.float32, tag="ps", name="pd_warm")
            for i in range(5):
                nc.tensor.matmul(pd[:], ut[:, 0, 0, 0:P],
                                 ut[:, 1, 7, 512:1024],
                                 start=(i == 0), stop=(i == 4))

            bn_affine(1, pk1, ar_u1, gate1, AB1, gA1)

            # dada2 pools via identity: p2 = gA1*pool(u') + pool(x)
            # (+64*B1 shift per channel cancels inside the dada BN)
            for ci in range(CI):
                for n in range(NIMG):
                    nc.vector.scalar_tensor_tensor(
                        p_tiles[2][:, ci, n, :], pools_u[:, ci, n, :],
                        gA1[:, ci, n:n + 1], p_tiles[1][:, ci, n, :],
                        ALU.mult, ALU.add)
            dada_block(2, dwt2, pk2, p_tiles[2], gate2, ystat2, ar_y2)

            # x1 = gA1[n]*u' + x (B1 folded into sign bias / final affine)
            for n in range(NIMG):
                for ci in range(CI):
                    tmp = tmppool.tile([P, S], DT.float32, tag="tmp",
                                       name=f"tmid_{n}_{ci}")
                    nc.vector.tensor_scalar(tmp[:], ut[:, ci, n, :],
                                            gA1[:, ci, n:n + 1], None,
                                            ALU.mult)
                    nc.gpsimd.tensor_add(xt[:, n, ci, :], tmp[:],
                                         xt[:, n, ci, :])
                    sign_into_spad(n, ci, bias=AB1[:, 1, ci:ci + 1])

            # ================= block 2 =================
            for oi in range(2):
                for imgs in QUADS:
                    for half in range(2):
                        conv_quad(2, w2sb, pk2, oi, imgs, half)

            main_stats(2, gate2, ustat2)
            allreduce_stats(ustat2, ar_u2, 2, "u")
            bn_affine(2, pk2, ar_u2, gate2, AB2, gA2)
            # B' = B1 + B2 (skip path carries the un-shifted v = x1 - B1)
            nc.vector.tensor_add(Bp[:], AB1[:, 1, :], AB2[:, 1, :])

            # out = gA2[n]*u' + B' + v ; 3-engine split + immediate DMA
            for n in range(NIMG):
                for ci in range(CI):
                    idx = n * 2 + ci
                    ov = out_t[n].rearrange("(ci p) s -> p ci s", p=P)
                    tmp = tmppool.tile([P, S], DT.float32, tag="tmp",
                                       name=f"tout_{n}_{ci}")
                    if idx % 8 < 5:
                        nc.scalar.activation(tmp[:], ut[:, ci, n, :],
                                             AF.Identity,
                                             bias=Bp[:, ci:ci + 1],
                                             scale=gA2[:, ci, n:n + 1])
                    else:
                        nc.vector.tensor_scalar(tmp[:], ut[:, ci, n, :],
                                                gA2[:, ci, n:n + 1],
                                                Bp[:, ci:ci + 1],
                                                ALU.mult, ALU.add)
                    eng = nc.gpsimd if idx % 2 == 0 else nc.vector
                    eng.tensor_add(xt[:, n, ci, :], tmp[:], xt[:, n, ci, :])
                    nc.sync.dma_start(ov[:, ci, :], xt[:, n, ci, :])

    nc.compile()
    return nc


def _pack_w(w):
    ws = np.sign(w.astype(np.float32))
    t = ws.reshape(2, P, CI, P, 3, 3)           # oi, o_lo, ci, c_lo, dy, dx
    t = t.transpose(3, 2, 4, 5, 0, 1)           # c_lo, ci, dy, dx, oi, o_lo
    return np.ascontiguousarray(t.reshape(P, CI, 9, 2, P)).astype(
        ml_dtypes.float8_e4m3)


def _pack_dw(dw):
    d = (dw.astype(np.float32) / 64.0).reshape(2, P, CI, P)  # oi,o_lo,ci,c_lo
    d = d.transpose(3, 2, 0, 1)                               # c_lo,ci,oi,o_lo
    hi = d.astype(ml_dtypes.bfloat16)
    lo = (d - hi.astype(np.float32)).astype(ml_dtypes.bfloat16)
    out = np.empty((P, CI, 2, 2, P), ml_dtypes.bfloat16)
    out[:, :, 0] = hi
    out[:, :, 1] = lo
    return out


def _pack_pk(w, a, g, b, dg, db):
    alpha = np.abs(w.astype(np.float32)).mean(axis=(1, 2, 3))
    fields = [alpha, a, g, b, dg, db]
    pk = np.empty((P, 6, CI), np.float32)
    for j, f in enumerate(fields):
        pk[:, j, :] = np.asarray(f, np.float32).reshape(CI, P).T
    return pk


def kernel(**inputs):
    if "nc" not in _CACHE:
        _CACHE["nc"] = _build()
    nc = _CACHE["nc"]

    x = np.asarray(inputs["x"], np.float32).reshape(64, 256, S)
    feed = {
        "w1sb": _pack_w(np.asarray(inputs["w1"])),
        "w2sb": _pack_w(np.asarray(inputs["w2"])),
        "dwt1": _pack_dw(np.asarray(inputs["dw1"])),
        "dwt2": _pack_dw(np.asarray(inputs["dw2"])),
        "pk1": _pack_pk(np.asarray(inputs["w1"]), inputs["a1"], inputs["g1"],
                        inputs["b1"], inputs["dg1"], inputs["db1"]),
        "pk2": _pack_pk(np.asarray(inputs["w2"]), inputs["a2"], inputs["g2"],
                        inputs["b2"], inputs["dg2"], inputs["db2"]),
    }
    in_maps = []
    for c in range(NCORES):
        m = dict(feed)
        m["x"] = np.ascontiguousarray(x[c * NIMG:(c + 1) * NIMG])
        in_maps.append(m)

    trace = bool(int(os.environ.get("BASS_KERNEL_TRACE", "0")))
    res = bass_utils.run_bass_kernel_spmd(
        nc, in_maps, core_ids=list(range(NCORES)), trace=trace)
    kernel.last_results = res

    out = np.concatenate([res.results[c]["out"] for c in range(NCORES)], axis=0)
    return out.reshape(64, 256, H, W)
